# revision 5
# baseline (speedup 1.0000x reference)
"""Cross-attention Trainium2 kernel.

Problem: B=4, S1=S2=2048, D1=D2=512, H=8, DK=DV=64.
  q = x1 @ Wq; k = x2 @ Wk; v = x3 @ Wv (per-head)
  out = softmax(q k^T / sqrt(64)) v, concat heads, @ Wo + bo

Sharding: 8 cores = (batch b in 0..3) x (head-group g in 0..1).
Each core handles one batch and 4 heads: column-parallel Wq/Wk/Wv,
row-parallel Wo. The two per-batch partial output projections are summed
on the host (plus bias) -- no on-device collectives.

Scores are in [-1.8, 1.8] (weights scaled by 0.02), so softmax is computed
without max-subtraction: exp row-sums <= ~2300, safe in fp32. The softmax
denominator comes for free from a ones-column appended to V.

All matmuls run in fp32r (full-rate on the PE for free dims >= 256).
"""

import os
import sys

for _p in ("/opt/trn_rl_repo", "/root/.axon_site/_ro/trn_rl_repo"):
    if os.path.isdir(_p) and _p not in sys.path:
        sys.path.insert(0, _p)

import numpy as np

import concourse.bass as bass
import concourse.bacc as bacc
import concourse.mybir as mybir
import concourse.tile as tile
from concourse import bass_utils
from concourse.masks import make_identity

B, S, D = 4, 2048, 512
H, DK, DV = 8, 64, 64
HPC = 4                # heads per core
GD = HPC * DK          # 256 head-dims per core
N_CORES = 8

F32 = mybir.dt.float32
F32R = mybir.dt.float32r
EXP = mybir.ActivationFunctionType.Exp


def build():
    nc = bacc.Bacc(
        "TRN2",
        target_bir_lowering=False,
        debug=False,
        enable_asserts=False,
        num_devices=N_CORES,
    )
    x1b = nc.dram_tensor("x1b", [S, D], F32, kind="ExternalInput").ap()
    x2b = nc.dram_tensor("x2b", [S, D], F32, kind="ExternalInput").ap()
    x3b = nc.dram_tensor("x3b", [S, D], F32, kind="ExternalInput").ap()
    wq = nc.dram_tensor("wq", [D, GD], F32, kind="ExternalInput").ap()
    wk = nc.dram_tensor("wk", [D, GD], F32, kind="ExternalInput").ap()
    wv = nc.dram_tensor("wv", [D, GD], F32, kind="ExternalInput").ap()
    wo = nc.dram_tensor("wo", [GD, D], F32, kind="ExternalInput").ap()
    out = nc.dram_tensor("out", [S, D], F32, kind="ExternalOutput").ap()

    with tile.TileContext(nc) as tc:
        with (
            tc.tile_pool(name="wpool", bufs=1) as wpool,
            tc.tile_pool(name="persist", bufs=1) as persist,
        ):
            # --- constants / weights -------------------------------------
            # DMA to f32 staging, then round-copy into f32r (the BIR verifier
            # requires fp32r matmul inputs to be produced as fp32r).
            wq_sb = wpool.tile([128, 4, GD], F32R)
            wk_sb = wpool.tile([128, 4, GD], F32R)
            wv_sb = wpool.tile([128, 4, GD], F32R)
            wo_sb = wpool.tile([128, 2, D], F32R)
            with tc.tile_pool(name="wstage", bufs=2) as wstage:
                for w_dram, w_dst, pat, kw in (
                    (wq, wq_sb, "(co ci) m -> ci co m", dict(ci=128)),
                    (wk, wk_sb, "(co ci) m -> ci co m", dict(ci=128)),
                    (wv, wv_sb, "(co ci) m -> ci co m", dict(ci=128)),
                    (wo, wo_sb, "(ho hi) d -> hi ho d", dict(hi=128)),
                ):
                    wst = wstage.tile(list(w_dst.shape), F32, tag="wst", name="wst")
                    nc.sync.dma_start(wst[:], w_dram.rearrange(pat, **kw))
                    nc.vector.tensor_copy(w_dst[:], wst[:])

            identity = wpool.tile([128, 128], F32)
            make_identity(nc, identity)
            ones_f = wpool.tile([128, 64], F32)
            nc.vector.memset(ones_f[:], 1.0)
            ones64 = wpool.tile([1, 64], F32R)
            nc.vector.tensor_copy(ones64[:], ones_f[0:1, :])
            zbias = wpool.tile([128, 1], F32)
            nc.vector.memset(zbias[:], 0.0)

            # --- persistent activations ----------------------------------
            # qT[p, hp, q]  : p = (h%2)*64 + dk, head h = 2*hp + p//64
            # kT[p, hp, ko, k] : same head layout, k = 128*ko + k_in
            # v_aug[p, ko, h, e] : p = k_in, e in [0,64) = dv, e = 64 -> 1.0
            # av_lhsT[p, hp, q] : p = (h%2)*64 + dv  (matches wo_sb rows)
            qT = persist.tile([128, 2, S], F32R)
            kT = persist.tile([128, 2, 16, 128], F32R)
            v_aug = persist.tile([128, 16, HPC, 65], F32R)
            av_lhsT = persist.tile([128, 2, S], F32R)
            nc.vector.tensor_copy(
                v_aug[:, :, :, 64:65],
                ones_f[:, 0:64].rearrange("p (a b c) -> p a b c", a=16, b=4),
            )

            # --- phase A: transpose inputs + projections -----------------
            with (
                tc.tile_pool(name="xin", bufs=3) as xin_pool,
                tc.tile_pool(name="xT", bufs=2) as xT_pool,
                tc.tile_pool(name="psA_t", bufs=4, space="PSUM") as psA_t,
                tc.tile_pool(name="psA_p", bufs=4, space="PSUM") as psA_p,
            ):

                def transpose_chunk(xsrc, so):
                    """Load 512 rows of x and produce xT chunk [128, 4, 512]:
                    xTc[ci, co, s] = x[so*512 + s, co*128 + ci]."""
                    xTc = xT_pool.tile([128, 4, 512], F32R, tag="xT", name="xTc")
                    for si in range(4):
                        xin = xin_pool.tile([128, D], F32, tag="xin", name="xin")
                        r0 = so * 512 + si * 128
                        nc.sync.dma_start(xin[:], xsrc[r0 : r0 + 128, :])
                        for co in range(4):
                            pt = psA_t.tile([128, 128], F32, tag="t", name="ptr")
                            nc.tensor.transpose(
                                pt[:], xin[:, co * 128 : (co + 1) * 128], identity[:]
                            )
                            nc.vector.tensor_copy(
                                xTc[:, co, si * 128 : (si + 1) * 128], pt[:]
                            )
                    return xTc

                # x1 -> qT
                for so in range(4):
                    xTc = transpose_chunk(x1b, so)
                    for hp in range(2):
                        pq = psA_p.tile([128, 512], F32, tag="p", name="pq")
                        for co in range(4):
                            nc.tensor.matmul(
                                pq[:],
                                (wq_sb[:, co, hp * 128 : (hp + 1) * 128]),
                                (xTc[:, co, :]),
                                start=(co == 0),
                                stop=(co == 3),
                            )
                        nc.vector.tensor_copy(
                            qT[:, hp, so * 512 : (so + 1) * 512], pq[:]
                        )

                # x2 -> kT
                for so in range(4):
                    xTc = transpose_chunk(x2b, so)
                    for hp in range(2):
                        pk = psA_p.tile([128, 512], F32, tag="p", name="pk")
                        for co in range(4):
                            nc.tensor.matmul(
                                pk[:],
                                (wk_sb[:, co, hp * 128 : (hp + 1) * 128]),
                                (xTc[:, co, :]),
                                start=(co == 0),
                                stop=(co == 3),
                            )
                        nc.vector.tensor_copy(
                            kT[:, hp, so * 4 : (so + 1) * 4, :],
                            pk[:].rearrange("p (a b) -> p a b", a=4),
                        )

                # x3 -> v_aug
                for so in range(4):
                    xTc = transpose_chunk(x3b, so)
                    for ks in range(4):
                        ko = so * 4 + ks
                        pv = psA_p.tile([128, GD], F32, tag="p", name="pv")
                        for co in range(4):
                            nc.tensor.matmul(
                                pv[:],
                                (xTc[:, co, ks * 128 : (ks + 1) * 128]),
                                (wv_sb[:, co, :]),
                                start=(co == 0),
                                stop=(co == 3),
                            )
                        nc.vector.tensor_copy(
                            v_aug[:, ko, :, 0:64],
                            pv[:].rearrange("p (h e) -> p h e", h=4),
                        )

            # --- phase B: attention --------------------------------------
            with (
                tc.tile_pool(name="psB", bufs=2, space="PSUM") as psB,
                tc.tile_pool(name="psAV", bufs=2, space="PSUM") as psAV,
                tc.tile_pool(name="ptp", bufs=3) as pt_pool,
                tc.tile_pool(name="ev", bufs=2) as ev_pool,
            ):
                for h in range(HPC):
                    hp, prow = h // 2, (h % 2) * 64
                    for qh in range(2):
                        qbase = qh * 1024
                        pav = psAV.tile([65, 1024], F32, tag="av", name="pav")
                        for ko in range(16):
                            ps = psB.tile([128, 1024], F32, tag="s", name="ps")
                            for j in range(2):
                                nc.tensor.matmul(
                                    ps[:, j * 512 : (j + 1) * 512],
                                    (kT[prow : prow + 64, hp, ko, :]),
                                    (
                                        qT[
                                            prow : prow + 64,
                                            hp,
                                            qbase + j * 512 : qbase + (j + 1) * 512,
                                        ]
                                    ),
                                    start=True,
                                    stop=True,
                                )
                            pt = pt_pool.tile([128, 1024], F32R, tag="pt", name="pt")
                            nc.scalar.activation(
                                pt[:], ps[:], EXP, bias=zbias[:], scale=0.125
                            )
                            for j in range(2):
                                nc.tensor.matmul(
                                    pav[:, j * 512 : (j + 1) * 512],
                                    (v_aug[:, ko, h, :]),
                                    (pt[:, j * 512 : (j + 1) * 512]),
                                    start=(ko == 0),
                                    stop=(ko == 15),
                                )
                        # eviction: normalize by the ones-row denominator
                        av_sb = ev_pool.tile([65, 1024], F32, tag="avsb", name="av_sb")
                        nc.vector.tensor_copy(av_sb[:], pav[:])
                        linv_f = ev_pool.tile([1, 1024], F32, tag="linvf", name="linv_f")
                        nc.vector.reciprocal(linv_f[:], av_sb[64:65, :])
                        linv = ev_pool.tile([1, 1024], F32R, tag="linv", name="linv")
                        nc.vector.tensor_copy(linv[:], linv_f[:])
                        bc = psB.tile([64, 1024], F32, tag="s", name="bc")
                        for j in range(2):
                            nc.tensor.matmul(
                                bc[:, j * 512 : (j + 1) * 512],
                                (ones64[:]),
                                (linv[:, j * 512 : (j + 1) * 512]),
                                start=True,
                                stop=True,
                            )
                        nc.vector.tensor_mul(
                            av_lhsT[prow : prow + 64, hp, qbase : qbase + 1024],
                            bc[:],
                            av_sb[0:64, :],
                        )

            # --- phase C: output projection ------------------------------
            with (
                tc.tile_pool(name="psC", bufs=2, space="PSUM") as psC,
                tc.tile_pool(name="osb", bufs=3) as osb_pool,
            ):
                for qt in range(16):
                    po = psC.tile([128, D], F32, tag="o", name="po")
                    for hp2 in range(2):
                        nc.tensor.matmul(
                            po[:],
                            (av_lhsT[:, hp2, qt * 128 : (qt + 1) * 128]),
                            (wo_sb[:, hp2, :]),
                            start=(hp2 == 0),
                            stop=(hp2 == 1),
                        )
                    ob = osb_pool.tile([128, D], F32, tag="ob", name="ob")
                    nc.vector.tensor_copy(ob[:], po[:])
                    nc.sync.dma_start(out[qt * 128 : (qt + 1) * 128, :], ob[:])

    nc.compile()
    return nc


_COMPILED = None


def _get_compiled():
    global _COMPILED
    if _COMPILED is None:
        _COMPILED = build()
    return _COMPILED


def _in_maps(x1, x2, x3, Wq, Wk, Wv, Wo):
    maps = []
    for b in range(B):
        xs = {
            "x1b": np.ascontiguousarray(np.asarray(x1[b], dtype=np.float32)),
            "x2b": np.ascontiguousarray(np.asarray(x2[b], dtype=np.float32)),
            "x3b": np.ascontiguousarray(np.asarray(x3[b], dtype=np.float32)),
        }
        for g in range(2):
            c0, c1 = g * GD, (g + 1) * GD
            maps.append(
                dict(
                    xs,
                    wq=np.ascontiguousarray(np.asarray(Wq[:, c0:c1], dtype=np.float32)),
                    wk=np.ascontiguousarray(np.asarray(Wk[:, c0:c1], dtype=np.float32)),
                    wv=np.ascontiguousarray(np.asarray(Wv[:, c0:c1], dtype=np.float32)),
                    wo=np.ascontiguousarray(np.asarray(Wo[c0:c1, :], dtype=np.float32)),
                )
            )
    return maps


def run(x1, x2, x3, Wq, Wk, Wv, Wo, bo, **spmd_kwargs):
    nc = _get_compiled()
    res = bass_utils.run_bass_kernel_spmd(
        nc, _in_maps(x1, x2, x3, Wq, Wk, Wv, Wo),
        core_ids=list(range(N_CORES)), **spmd_kwargs,
    )
    bo = np.asarray(bo, dtype=np.float32)
    out = np.empty((B, S, D), dtype=np.float32)
    for b in range(B):
        out[b] = res.results[2 * b]["out"] + res.results[2 * b + 1]["out"] + bo
    return out, res


def kernel(x1, x2, x3, Wq, Wk, Wv, Wo, bo):
    out, _ = run(x1, x2, x3, Wq, Wk, Wv, Wo, bo)
    return out


# revision 12
# speedup vs baseline: 1.0451x; 1.0451x over previous
"""Cross-attention Trainium2 kernel.

Problem: B=4, S1=S2=2048, D1=D2=512, H=8, DK=DV=64.
  q = x1 @ Wq; k = x2 @ Wk; v = x3 @ Wv (per-head)
  out = softmax(q k^T / sqrt(64)) v, concat heads, @ Wo + bo

Sharding: 8 cores = (batch b in 0..3) x (head-group g in 0..1).
Each core handles one batch and 4 heads: column-parallel Wq/Wk/Wv,
row-parallel Wo. The two per-batch partial output projections are summed
on the host (plus bias) -- no on-device collectives.

Scores are in [-1.8, 1.8] (weights scaled by 0.02), so softmax is computed
without max-subtraction: exp row-sums <= ~2300, safe in fp32. The softmax
denominator comes for free from a ones-column appended to V.

All matmuls run in fp32r (full-rate on the PE for free dims >= 256).
Per-core engine budget (cost model): ACT ~128us of exp (the floor),
PE ~150us, DVE ~70us. Structure: phase A (transpose+projections,
DVE/PE-bound), phase B (attention, ACT-bound) with the output projection
of each q-half interleaved under the other half's attention.
"""

import os
import sys

for _p in ("/opt/trn_rl_repo", "/root/.axon_site/_ro/trn_rl_repo"):
    if os.path.isdir(_p) and _p not in sys.path:
        sys.path.insert(0, _p)

import numpy as np

import concourse.bass as bass
import concourse.bacc as bacc
import concourse.mybir as mybir
import concourse.tile as tile
from concourse import bass_utils
from concourse.masks import make_identity

B, S, D = 4, 2048, 512
H, DK, DV = 8, 64, 64
HPC = 4                # heads per core
GD = HPC * DK          # 256 head-dims per core
N_CORES = 8

F32 = mybir.dt.float32
F32R = mybir.dt.float32r
EXP = mybir.ActivationFunctionType.Exp


PHASES = "all"   # diagnostics: "a" = phase A only, "bc" = attention+output only


def build(n_iters: int = 1):
    nc = bacc.Bacc(
        "TRN2",
        target_bir_lowering=False,
        debug=False,
        enable_asserts=False,
        num_devices=N_CORES,
    )
    x1b = nc.dram_tensor("x1b", [S, D], F32, kind="ExternalInput").ap()
    x2b = nc.dram_tensor("x2b", [S, D], F32, kind="ExternalInput").ap()
    x3b = nc.dram_tensor("x3b", [S, D], F32, kind="ExternalInput").ap()
    wq = nc.dram_tensor("wq", [D, GD], F32, kind="ExternalInput").ap()
    wk = nc.dram_tensor("wk", [D, GD], F32, kind="ExternalInput").ap()
    wv = nc.dram_tensor("wv", [D, GD], F32, kind="ExternalInput").ap()
    wo = nc.dram_tensor("wo", [GD, D], F32, kind="ExternalInput").ap()
    out = nc.dram_tensor("out", [S, D], F32, kind="ExternalOutput").ap()

    with tile.TileContext(nc) as tc:
        for _it in range(n_iters):
            _emit_iteration(nc, tc, x1b, x2b, x3b, wq, wk, wv, wo, out)
    nc.compile()
    return nc


def _emit_iteration(nc, tc, x1b, x2b, x3b, wq, wk, wv, wo, out):
    with (
        tc.tile_pool(name="wpool", bufs=1) as wpool,
        tc.tile_pool(name="persist", bufs=1) as persist,
    ):
        # --- constants / weights -----------------------------------------
        # DMA to f32 staging, then round-copy into f32r (the BIR verifier
        # requires fp32r matmul inputs to be produced as fp32r). Copies go
        # on ScalarE, which is idle until the attention phase.
        wq_sb = wpool.tile([128, 4, GD], F32R)
        wk_sb = wpool.tile([128, 4, GD], F32R)
        wv_sb = wpool.tile([128, 4, GD], F32R)
        wo_sb = wpool.tile([128, 2, D], F32R)

        def stage_weights(wstage):
            # emitted inside phase A after the first x-chunk loads, so the
            # weight DMAs don't delay the x stream on the DMA queues
            for w_dram, w_dst, pat, kw in (
                (wq, wq_sb, "(co ci) m -> ci co m", dict(ci=128)),
                (wk, wk_sb, "(co ci) m -> ci co m", dict(ci=128)),
                (wv, wv_sb, "(co ci) m -> ci co m", dict(ci=128)),
                (wo, wo_sb, "(ho hi) d -> hi ho d", dict(hi=128)),
            ):
                wst = wstage.tile(list(w_dst.shape), F32, tag="wst", name="wst")
                nc.sync.dma_start(wst[:], w_dram.rearrange(pat, **kw))
                nc.scalar.copy(w_dst[:], wst[:])

        identity = wpool.tile([128, 128], F32)
        make_identity(nc, identity)
        ones_f = wpool.tile([128, 64], F32)
        nc.vector.memset(ones_f[:], 1.0)
        ones64 = wpool.tile([1, 64], F32R)
        nc.vector.tensor_copy(ones64[:], ones_f[0:1, :])
        zbias = wpool.tile([128, 1], F32)
        nc.vector.memset(zbias[:], 0.0)

        # --- persistent activations --------------------------------------
        # qT[p, hp, q]  : p = (h%2)*64 + dk, head h = 2*hp + p//64
        # kT[p, hp, ko, k] : same head layout, k = 128*ko + k_in
        # v_aug[p, ko, h, e] : p = k_in, e in [0,64) = dv, e = 64 -> 1.0
        # av_lhsT[p, hp, q] : p = (h%2)*64 + dv  (matches wo_sb rows)
        qT = persist.tile([128, 2, S], F32R)
        kT = persist.tile([128, 2, 16, 128], F32R)
        v_aug = persist.tile([128, 16, HPC, 65], F32R)
        av_lhsT = persist.tile([128, 2, S], F32R)
        nc.vector.tensor_copy(
            v_aug[:, :, :, 64:65],
            ones_f[:, 0:64].rearrange("p (a b c) -> p a b c", a=16, b=4),
        )

        if PHASES in ("all", "a"):
            _emit_phase_a(nc, tc, x1b, x2b, x3b, wq_sb, wk_sb, wv_sb,
                          identity, qT, kT, v_aug, stage_weights)
        if PHASES in ("all", "bc"):
            _emit_phase_bc(nc, tc, qT, kT, v_aug, av_lhsT, wo_sb,
                           ones64, zbias, out)


def _emit_phase_a(nc, tc, x1b, x2b, x3b, wq_sb, wk_sb, wv_sb,
                  identity, qT, kT, v_aug, stage_weights):
        # --- phase A: transpose inputs + projections ---------------------
        # Software-pipelined: while chunk n is being PE-transposed, chunk
        # n-1's projection matmuls run, so the in-order PE stream never
        # waits on the DVE eviction of the chunk it just transposed.
        with (
            tc.tile_pool(name="xin", bufs=8) as xin_pool,
            tc.tile_pool(name="xT", bufs=3) as xT_pool,
            tc.tile_pool(name="psA_t", bufs=4, space="PSUM") as psA_t,
            tc.tile_pool(name="psA_p", bufs=4, space="PSUM") as psA_p,
            tc.tile_pool(name="wstage", bufs=2) as wstage_pool,
        ):

            def transpose_chunk(xsrc, so):
                """Load 512 rows of x and produce xT chunk [128, 4, 512]:
                xTc[ci, co, s] = x[so*512 + s, co*128 + ci].
                4 PE transposes batched into one PSUM bank -> one DVE copy."""
                xTc = xT_pool.tile([128, 4, 512], F32R, tag="xT", name="xTc")
                for si in range(4):
                    xin = xin_pool.tile([128, D], F32, tag="xin", name="xin")
                    r0 = so * 512 + si * 128
                    nc.sync.dma_start(xin[:], xsrc[r0 : r0 + 128, :])
                    ptb = psA_t.tile([128, 4, 128], F32, tag="t", name="ptb")
                    for co in range(4):
                        nc.tensor.transpose(
                            ptb[:, co, :], xin[:, co * 128 : (co + 1) * 128],
                            identity[:],
                        )
                    if si % 2 == 0:
                        nc.vector.tensor_copy(
                            xTc[:, :, si * 128 : (si + 1) * 128], ptb[:]
                        )
                    else:
                        nc.scalar.copy(
                            xTc[:, :, si * 128 : (si + 1) * 128], ptb[:]
                        )
                return xTc

            def project(kind, so, xTc):
                if kind in ("q", "k"):
                    w_sb = wq_sb if kind == "q" else wk_sb
                    for hp in range(2):
                        pq = psA_p.tile([128, 512], F32, tag="p", name="pq")
                        for co in range(4):
                            nc.tensor.matmul(
                                pq[:],
                                w_sb[:, co, hp * 128 : (hp + 1) * 128],
                                xTc[:, co, :],
                                start=(co == 0),
                                stop=(co == 3),
                            )
                        if kind == "q":
                            nc.scalar.copy(
                                qT[:, hp, so * 512 : (so + 1) * 512], pq[:]
                            )
                        else:
                            nc.scalar.copy(
                                kT[:, hp, so * 4 : (so + 1) * 4, :],
                                pq[:].rearrange("p (a b) -> p a b", a=4),
                            )
                else:
                    for ks in range(4):
                        ko = so * 4 + ks
                        pv = psA_p.tile([128, GD], F32, tag="p", name="pv")
                        for co in range(4):
                            nc.tensor.matmul(
                                pv[:],
                                xTc[:, co, ks * 128 : (ks + 1) * 128],
                                wv_sb[:, co, :],
                                start=(co == 0),
                                stop=(co == 3),
                            )
                        nc.scalar.copy(
                            v_aug[:, ko, :, 0:64],
                            pv[:].rearrange("p (h e) -> p h e", h=4),
                        )

            chunks = (
                [(x1b, so, "q") for so in range(4)]
                + [(x2b, so, "k") for so in range(4)]
                + [(x3b, so, "v") for so in range(4)]
            )
            prev = None
            for ci, (xsrc, so, kind) in enumerate(chunks):
                xTc = transpose_chunk(xsrc, so)
                if ci == 0:
                    stage_weights(wstage_pool)
                if prev is not None:
                    project(prev[0], prev[1], prev[2])
                prev = (kind, so, xTc)
            project(prev[0], prev[1], prev[2])


def _emit_phase_bc(nc, tc, qT, kT, v_aug, av_lhsT, wo_sb, ones64, zbias, out):
        # --- phase B: attention, with phase C (output projection) of each
        # q-half interleaved under the other half's ACT-bound attention ----
        S_ = S
        with (
            tc.tile_pool(name="psB", bufs=2, space="PSUM") as psB,
            tc.tile_pool(name="psAV", bufs=1, space="PSUM") as psAV,
            tc.tile_pool(name="psC", bufs=2, space="PSUM") as psC,
            tc.tile_pool(name="ptp", bufs=4) as pt_pool,
            tc.tile_pool(name="ev", bufs=2) as ev_pool,
            tc.tile_pool(name="osb", bufs=3) as osb_pool,
        ):
            ev_pending = None
            for qh in range(2):
                qbase = qh * 1024

                def normalize(av_sb, hp, prow, qbase):
                    linv_f = ev_pool.tile([1, 1024], F32, tag="linvf", name="linv_f")
                    nc.vector.reciprocal(linv_f[:], av_sb[64:65, :])
                    linv = ev_pool.tile([1, 1024], F32R, tag="linv", name="linv")
                    nc.vector.tensor_copy(linv[:], linv_f[:])
                    bc = psB.tile([64, 1024], F32, tag="s", name="bc")
                    for j in range(2):
                        nc.tensor.matmul(
                            bc[:, j * 512 : (j + 1) * 512],
                            ones64[:],
                            linv[:, j * 512 : (j + 1) * 512],
                            start=True,
                            stop=True,
                        )
                    nc.vector.tensor_mul(
                        av_lhsT[prow : prow + 64, hp, qbase : qbase + 1024],
                        bc[:],
                        av_sb[0:64, :],
                    )

                for h in range(HPC):
                    hp, prow = h // 2, (h % 2) * 64
                    pav = psAV.tile([65, 1024], F32, tag="av", name="pav")

                    def av_mm(pt_prev, ko_prev):
                        for j in range(2):
                            nc.tensor.matmul(
                                pav[:, j * 512 : (j + 1) * 512],
                                v_aug[:, ko_prev, h, :],
                                pt_prev[:, j * 512 : (j + 1) * 512],
                                start=(ko_prev == 0),
                                stop=(ko_prev == 15),
                            )

                    pending = None
                    for ko in range(16):
                        ps = psB.tile([128, 1024], F32, tag="s", name="ps")
                        for j in range(2):
                            nc.tensor.matmul(
                                ps[:, j * 512 : (j + 1) * 512],
                                kT[prow : prow + 64, hp, ko, :],
                                qT[
                                    prow : prow + 64,
                                    hp,
                                    qbase + j * 512 : qbase + (j + 1) * 512,
                                ],
                                start=True,
                                stop=True,
                            )
                        pt = pt_pool.tile([128, 1024], F32R, tag="pt", name="pt")
                        nc.scalar.activation(
                            pt[:], ps[:], EXP, bias=zbias[:], scale=0.125
                        )
                        if pending is not None:
                            av_mm(*pending)
                        pending = (pt, ko)
                    av_mm(*pending)
                    # stage 1 of eviction: copy PSUM av (+denominator row) out
                    # so pav can be reused by the next head immediately.
                    av_sb = ev_pool.tile([65, 1024], F32, tag="avsb", name="av_sb")
                    nc.vector.tensor_copy(av_sb[:], pav[:])
                    # stage 2 (recip + broadcast-matmul + normalize) is
                    # deferred one head so the PE never stalls on the DVE
                    # eviction chain at a head boundary.
                    if ev_pending is not None:
                        normalize(*ev_pending)
                    ev_pending = (av_sb, hp, prow, qbase)
                # flush the last head's normalization before phase C needs it
                normalize(*ev_pending)
                ev_pending = None

                # phase C for this q-half (overlaps the other half's phase B)
                for qt in range(qh * 8, qh * 8 + 8):
                    po = psC.tile([128, D], F32, tag="o", name="po")
                    for hp2 in range(2):
                        nc.tensor.matmul(
                            po[:],
                            av_lhsT[:, hp2, qt * 128 : (qt + 1) * 128],
                            wo_sb[:, hp2, :],
                            start=(hp2 == 0),
                            stop=(hp2 == 1),
                        )
                    ob = osb_pool.tile([128, D], F32, tag="ob", name="ob")
                    nc.vector.tensor_copy(ob[:], po[:])
                    nc.sync.dma_start(out[qt * 128 : (qt + 1) * 128, :], ob[:])


_COMPILED = None


def _get_compiled():
    global _COMPILED
    if _COMPILED is None:
        _COMPILED = build()
    return _COMPILED


def _in_maps(x1, x2, x3, Wq, Wk, Wv, Wo):
    maps = []
    for b in range(B):
        xs = {
            "x1b": np.ascontiguousarray(np.asarray(x1[b], dtype=np.float32)),
            "x2b": np.ascontiguousarray(np.asarray(x2[b], dtype=np.float32)),
            "x3b": np.ascontiguousarray(np.asarray(x3[b], dtype=np.float32)),
        }
        for g in range(2):
            c0, c1 = g * GD, (g + 1) * GD
            maps.append(
                dict(
                    xs,
                    wq=np.ascontiguousarray(np.asarray(Wq[:, c0:c1], dtype=np.float32)),
                    wk=np.ascontiguousarray(np.asarray(Wk[:, c0:c1], dtype=np.float32)),
                    wv=np.ascontiguousarray(np.asarray(Wv[:, c0:c1], dtype=np.float32)),
                    wo=np.ascontiguousarray(np.asarray(Wo[c0:c1, :], dtype=np.float32)),
                )
            )
    return maps


def run(x1, x2, x3, Wq, Wk, Wv, Wo, bo, **spmd_kwargs):
    nc = _get_compiled()
    res = bass_utils.run_bass_kernel_spmd(
        nc, _in_maps(x1, x2, x3, Wq, Wk, Wv, Wo),
        core_ids=list(range(N_CORES)), **spmd_kwargs,
    )
    bo = np.asarray(bo, dtype=np.float32)
    out = np.empty((B, S, D), dtype=np.float32)
    for b in range(B):
        out[b] = res.results[2 * b]["out"] + res.results[2 * b + 1]["out"] + bo
    return out, res


def kernel(x1, x2, x3, Wq, Wk, Wv, Wo, bo):
    out, _ = run(x1, x2, x3, Wq, Wk, Wv, Wo, bo)
    return out
